# revision 23
# baseline (speedup 1.0000x reference)
"""Trainium2 Bass kernel for blended-expert MLP (moe_routing).

Model: for each of 3 layers, w_l = sum_e c_e * W[l,e]  (E=8 experts),
x = act(x @ w_l.T + B_l), act = ELU for layers 0,1, none for layer 2.

Strategy (8 NeuronCores):
- Data-parallel over the 4096-row batch (512 rows/core).
- The expert blend is sharded over the contraction (input) dim: core k blends
  i-slice k (256 rows of w_l.T) on the Vector engine, then an 8-core
  AllGather assembles the full transposed blended weight w_l.T (2048x2048)
  in DRAM, which the matmul phase streams as stationary operands.
- Matmuls run in bf16 (fp32 PSUM accumulation); expert weights and x are
  uploaded as bf16 too, halving the dominant HBM streams (rel err ~5e-3).
  Activations stay SBUF-resident between layers in [feature, batch]
  orientation; ELU is fused DVE/ACT ops: max(exp(min(z,0))-1, z).
- Each layer's AllGather is split into two COLUMN halves aligned with the
  matmul m-loop, so matmuls on the first half overlap the second half's
  gather; a 32-byte warmup AllGather eats the per-execution first-CC-op
  cost so AG0a runs at full rate. The matmul loop runs in quarter-groups
  of 4 m-tiles on alternating PSUM bank sets, letting each group's
  bias/ELU epilogue drain under the next group's matmuls.
- kernel() runs the NEFF once with the real inputs and discards the
  result — the throwaway execution absorbs CC firmware staging (and any
  first-execution AllGather corruption), so the returned result comes
  from a clean, already-staged execution.
- Host side only reshapes/transposes/slices for sharding and assembles the
  output; all FLOPs (blend, matmul, bias, ELU) run on device.
"""

import numpy as np

import concourse.mybir as mybir
import concourse.tile as tile
from concourse import bacc
from concourse.bass_utils import run_bass_kernel_spmd

N_CORES = 8
L = 3          # layers
E = 8          # experts
D = 2048       # feature dim
BATCH = 4096
BS = BATCH // N_CORES   # 512 batch rows per core
IS = D // N_CORES       # 256 contraction rows blended per core
KT = D // 128           # 16 k-tiles
MT = D // 128           # 16 m-tiles (output feature tiles)
HALF_M = MT // 2        # 8 psum banks per half

f32 = mybir.dt.float32
f32r = mybir.dt.float32r
bf16 = mybir.dt.bfloat16

USE_BF16 = True

_cache: dict = {}


def _build():
    nc = bacc.Bacc("TRN2", target_bir_lowering=False, debug=False,
                   num_devices=N_CORES)
    indt = bf16 if USE_BF16 else f32
    # Per-core inputs (pre-sharded/transposed by host):
    # WtT: (L, E, IS, D) = this core's i-slice of W transposed to [in, out]
    WtT = nc.dram_tensor("WtT", [L, E, IS, D], indt, kind="ExternalInput")
    # xT: (D, BS) = this core's batch shard, transposed
    xT = nc.dram_tensor("xT", [D, BS], indt, kind="ExternalInput")
    # cb: (128, E) = weight_blend broadcast across partitions
    cb = nc.dram_tensor("cb", [128, E], f32, kind="ExternalInput")
    # Bp: (128, L*MT) = biases; col l*MT+m holds B[l, m*128 : (m+1)*128]
    Bp = nc.dram_tensor("Bp", [128, L * MT], f32, kind="ExternalInput")
    # Output: yT (D, BS) = this core's output shard, transposed, bf16
    # (host converts back to f32; quantization adds ~2e-3 rel err).
    yT = nc.dram_tensor("yT", [D, BS], bf16, kind="ExternalOutput")

    with tile.TileContext(nc) as tc:
        with (
            tc.tile_pool(name="const", bufs=1) as cpool,
            tc.tile_pool(name="acts", bufs=1) as apool,
            tc.tile_pool(name="blend", bufs=1) as bpool,
            tc.tile_pool(name="exp", bufs=8) as epool,
            tc.tile_pool(name="wst", bufs=24) as wpool,
            tc.tile_pool(name="epi", bufs=4) as xpool,
            tc.tile_pool(name="psum", bufs=1, space="PSUM") as ppool,
            tc.tile_pool(name="dram", bufs=1, space="DRAM") as dram,
        ):
            cbt = cpool.tile([128, E], f32)
            bt = cpool.tile([128, L * MT], f32)
            nc.sync.dma_start(cbt[:], cb[:])
            nc.sync.dma_start(bt[:], Bp[:])

            # Tiny warmup AllGather: the first data-carrying CC op of an
            # execution pays a ~20us first-op cost (measured); burn it on
            # 32 bytes so AG0h0 runs at full rate. Its doorbell rings
            # immediately (only needs cb), so it also rides the runtime's
            # entry barrier. No completion-gating read: first-execution
            # staging corruption is absorbed by kernel()'s throwaway run.
            wuin = dram.tile([1, E], f32, name="wuin")
            wuout = dram.tile([N_CORES, E], f32, addr_space="Shared",
                              name="wuout")
            nc.sync.dma_start(wuin[:], cb[:1, :])
            nc.gpsimd.collective_compute(
                "AllGather", mybir.AluOpType.bypass,
                ins=[wuin.opt()], outs=[wuout.opt()],
                replica_groups=[list(range(N_CORES))],
            )

            mmdt = bf16 if USE_BF16 else f32r
            # Activations: two sets of KT tiles [128, BS], ping-pong.
            acts = [
                [apool.tile([128, BS], mmdt, name=f"act{s}_{k}")
                 for k in range(KT)]
                for s in range(2)
            ]
            # DRAM bounce buffers for the weight AllGathers. Each layer
            # gathers as two COLUMN halves (m 0:1024 / 1024:2048) that
            # align exactly with the matmul's m-half loop: half-AG a
            # lands ~30us before a monolithic AG would, so mm_l's first
            # m-half starts while half b is still gathering. This removes
            # the measured 9-15us stalls where monolithic AG1/AG2 ended
            # just after the matmuls needed them.
            # Layer 0 gathers in FOUR column quarters (rate-matched to
            # the ~17us quarter-group matmuls so mm0 never waits long on
            # the next chunk); layers 1-2 in two halves (already
            # stall-free, fewer ops = less CC overhead).
            NCH = [4, 2, 2]
            agins = [
                [dram.tile([IS, D // NCH[l]], mmdt, name=f"agin{l}_{ch}")
                 for ch in range(NCH[l])]
                for l in range(L)
            ]
            agouts = [
                [dram.tile([D, D // NCH[l]], mmdt, addr_space="Shared",
                           name=f"agout{l}_{ch}")
                 for ch in range(NCH[l])]
                for l in range(L)
            ]

            # ---- blend + AllGather emission, one layer at a time.
            # Layers 0/1 go up-front; layer 2's blend is emitted between
            # mm0 and mm1 so its expert-load DMA traffic stays out of
            # AllGather-1's window. ----
            def emit_blend(l, ldeng=None, ltag="exp", lbufs=None):
                engs = ldeng if isinstance(ldeng, list) else [ldeng or nc.scalar]
                for h in range(IS // 128):  # 2 half-slices of 128 partitions
                    acc = bpool.tile([128, D], f32, name=f"acc{l}_{h}",
                                     tag=f"acc{h}", bufs=2)
                    accq = bpool.tile([128, D], mmdt, name=f"accq{l}_{h}",
                                      tag=f"accq{h}", bufs=2)
                    for e in range(E):
                        kw = {} if lbufs is None else {"bufs": lbufs}
                        et = epool.tile([128, D], indt, name=f"exp{l}_{h}_{e}",
                                        tag=ltag, **kw)
                        engs[(h * E + e) % len(engs)].dma_start(
                            et[:], WtT[l, e, h * 128:(h + 1) * 128, :]
                        )
                        if e == 0:
                            nc.vector.tensor_scalar_mul(
                                acc[:], et[:], cbt[:, 0:1]
                            )
                        elif e < E - 1:
                            # acc = (et * c_e) + acc
                            nc.vector.scalar_tensor_tensor(
                                acc[:], et[:], cbt[:, e:e + 1], acc[:],
                                mybir.AluOpType.mult, mybir.AluOpType.add,
                            )
                        else:
                            # last expert writes the matmul-dtype copy
                            nc.vector.scalar_tensor_tensor(
                                accq[:], et[:], cbt[:, e:e + 1], acc[:],
                                mybir.AluOpType.mult, mybir.AluOpType.add,
                            )
                    cw = D // NCH[l]
                    for ch in range(NCH[l]):
                        nc.gpsimd.dma_start(
                            agins[l][ch][h * 128:(h + 1) * 128, :],
                            accq[:, ch * cw:(ch + 1) * cw],
                        )
                for ch in range(NCH[l]):
                    nc.gpsimd.collective_compute(
                        "AllGather", mybir.AluOpType.bypass,
                        ins=[agins[l][ch].opt()], outs=[agouts[l][ch].opt()],
                        replica_groups=[list(range(N_CORES))],
                    )

            emit_blend(0)
            # x loads go first on the sync queue: emitted later they land
            # inside AllGather-0's window and slow it (queue is FIFO).
            for k in range(KT):
                xsrc = xT[k * 128:(k + 1) * 128, :]
                nc.sync.dma_start(
                    acts[0][k][:], xsrc if USE_BF16 else xsrc.bitcast(f32r)
                )
            emit_blend(1, ldeng=nc.sync, ltag="exp1", lbufs=4)
            # blend2 emitted up-front too: its DVE ops must precede mm0's
            # epilogue ops in the Vector engine's in-order queue, else
            # AG2's doorbell waits on mm0's full epilogue (engine queues
            # are FIFO; DMA loads schedule eagerly either way).
            # blend2's loads split across both HWDGE queues: they queue
            # behind blend0 (scalar) / x+blend1 (sync) FIFO entries and
            # finish ~75us — before AllGather-0's window opens (~88us).
            # (gpsimd/SWDGE tried instead: too slow, loads leaked into
            # AG0's window and cost it ~18us.)
            emit_blend(2, ldeng=[nc.scalar, nc.sync], ltag="exp2", lbufs=4)

            for l in range(L):
                # ---- matmul: y_l.T[m,:] = sum_k w_l.T[k,m].T @ act[k] ----
                src = acts[l % 2]
                dst = acts[(l + 1) % 2]
                # Quarter-groups of 4 m-tiles on ALTERNATING psum bank
                # sets (0-3 / 4-7): group g+1's matmuls don't share banks
                # with group g, so g's epilogue drains under g+1's compute
                # instead of stalling the PE ~3us at every boundary.
                QM = HALF_M // 2
                for q in range(4):
                    cw = D // NCH[l]
                    ch, off = (q * 512) // cw, (q * 512) % cw
                    psums = [
                        ppool.tile([128, BS], f32, name=f"ps{l}_{q}_{m}",
                                   tag=f"bank{(q % 2) * QM + m}")
                        for m in range(QM)
                    ]
                    for k in range(KT):
                        ws = wpool.tile([128, QM * 128], mmdt,
                                        name=f"ws{l}_{q}_{k}", tag="ws")
                        # alternate ws loads across both HWDGE queues
                        # (scalar queue is idle after the head phase) to
                        # halve per-tile feed latency under AG contention
                        wse = nc.sync if k % 2 == 0 else nc.scalar
                        wse.dma_start(
                            ws[:],
                            agouts[l][ch][k * 128:(k + 1) * 128,
                                          off:off + QM * 128],
                        )
                        for m in range(QM):
                            nc.tensor.matmul(
                                psums[m][:],
                                ws[:, m * 128:(m + 1) * 128],
                                src[k][:],
                                start=(k == 0),
                                stop=(k == KT - 1),
                            )
                    # ---- epilogue: bias (+ ELU), write next-layer acts ----
                    for m in range(QM):
                        gm = q * QM + m
                        bias = bt[:, l * MT + gm: l * MT + gm + 1]
                        ps = psums[m]
                        if l < L - 1:
                            tt = xpool.tile([128, BS], f32,
                                            name=f"t{l}_{gm}", tag="tmin")
                            zt = xpool.tile([128, BS], f32,
                                            name=f"z{l}_{gm}", tag="zbias")
                            ut = xpool.tile([128, BS], f32,
                                            name=f"u{l}_{gm}", tag="uexp")
                            # t = min(psum + bias, 0) on DVE
                            nc.vector.tensor_scalar(
                                tt[:], ps[:], bias, 0.0,
                                mybir.AluOpType.add, mybir.AluOpType.min,
                            )
                            # z = psum + bias on ACT
                            nc.scalar.activation(
                                zt[:], ps[:],
                                mybir.ActivationFunctionType.Identity,
                                bias=bias,
                            )
                            # u = exp(t) on ACT
                            nc.scalar.activation(
                                ut[:], tt[:], mybir.ActivationFunctionType.Exp
                            )
                            # act_next = max(u - 1, z) on DVE, in matmul dtype
                            nc.vector.scalar_tensor_tensor(
                                dst[gm][:], ut[:], 1.0, zt[:],
                                mybir.AluOpType.subtract, mybir.AluOpType.max,
                            )
                        else:
                            ot = xpool.tile([128, BS], bf16,
                                            name=f"o{gm}", tag="outt")
                            # bias-add split across ACT and DVE so the
                            # final half's serial epilogue chain halves.
                            if gm % 2 == 0:
                                nc.scalar.activation(
                                    ot[:], ps[:],
                                    mybir.ActivationFunctionType.Identity,
                                    bias=bias,
                                )
                            else:
                                nc.vector.tensor_scalar_add(
                                    ot[:], ps[:], bias
                                )
                            # HWDGE store (~0.6us fixed vs ~2us SWDGE);
                            # alternate queues so the 16 stores drain in
                            # parallel at the tail.
                            steng = nc.scalar if gm % 2 == 0 else nc.gpsimd
                            steng.dma_start(
                                yT[gm * 128:(gm + 1) * 128, :], ot[:]
                            )
    nc.finalize()
    return nc


def _get_nc():
    if "nc" not in _cache:
        _cache["nc"] = _build()
    return _cache["nc"]


def make_in_maps(weight_blend, x, W, B):
    weight_blend = np.asarray(weight_blend, dtype=np.float32)
    x = np.asarray(x, dtype=np.float32)
    W = np.asarray(W, dtype=np.float32)
    B = np.asarray(B, dtype=np.float32)

    cb = np.ascontiguousarray(np.broadcast_to(weight_blend[None, :], (128, E)))
    # Bp[p, l*MT+m] = B[l, m*128+p]
    Bp = np.ascontiguousarray(
        B.reshape(L, MT, 128).transpose(2, 0, 1).reshape(128, L * MT)
    )

    import ml_dtypes
    indt = ml_dtypes.bfloat16 if USE_BF16 else np.float32
    in_maps = []
    for k in range(N_CORES):
        WtT = np.ascontiguousarray(
            W[:, :, :, k * IS:(k + 1) * IS].transpose(0, 1, 3, 2)
        ).astype(indt)
        xTk = np.ascontiguousarray(x[k * BS:(k + 1) * BS, :].T).astype(indt)
        in_maps.append({"WtT": WtT, "xT": xTk, "cb": cb, "Bp": Bp})
    return in_maps


def kernel(weight_blend, x, W, B) -> np.ndarray:
    in_maps = make_in_maps(weight_blend, x, W, B)
    nc = _get_nc()
    last_err = None
    for attempt in range(3):
        try:
            if "staged" not in _cache:
                # Throwaway execution: absorbs CC firmware staging — the
                # first execution's collectives can be corrupted by
                # doorbells ringing during staging, so never return it.
                run_bass_kernel_spmd(nc, in_maps,
                                     core_ids=list(range(N_CORES)))
                _cache["staged"] = True
            res = run_bass_kernel_spmd(nc, in_maps,
                                       core_ids=list(range(N_CORES)))
            out = np.empty((BATCH, D), dtype=np.float32)
            for k in range(N_CORES):
                out[k * BS:(k + 1) * BS, :] = res.results[k]["yT"].T.astype(
                    np.float32)
            if np.isfinite(out).all():
                return out
            last_err = RuntimeError("non-finite kernel output")
        except Exception as e:  # transient NRT/device wedge: retry
            last_err = e
        import time as _time
        _time.sleep(10 * (attempt + 1))
    raise last_err


# revision 24
# speedup vs baseline: 1.0247x; 1.0247x over previous
"""Trainium2 Bass kernel for blended-expert MLP (moe_routing).

Model: for each of 3 layers, w_l = sum_e c_e * W[l,e]  (E=8 experts),
x = act(x @ w_l.T + B_l), act = ELU for layers 0,1, none for layer 2.

Strategy (8 NeuronCores):
- Data-parallel over the 4096-row batch (512 rows/core).
- The expert blend is sharded over the contraction (input) dim: core k blends
  i-slice k (256 rows of w_l.T) on the Vector engine, then an 8-core
  AllGather assembles the full transposed blended weight w_l.T (2048x2048)
  in DRAM, which the matmul phase streams as stationary operands.
- Matmuls run in bf16 (fp32 PSUM accumulation); expert weights and x are
  uploaded as bf16 too, halving the dominant HBM streams (rel err ~5e-3).
  Activations stay SBUF-resident between layers in [feature, batch]
  orientation; ELU is fused DVE/ACT ops: max(exp(min(z,0))-1, z).
- Each layer's AllGather is split into two COLUMN halves aligned with the
  matmul m-loop, so matmuls on the first half overlap the second half's
  gather; a 32-byte warmup AllGather eats the per-execution first-CC-op
  cost so AG0a runs at full rate. The matmul loop runs in quarter-groups
  of 4 m-tiles on alternating PSUM bank sets, letting each group's
  bias/ELU epilogue drain under the next group's matmuls.
- kernel() runs the NEFF once with the real inputs and discards the
  result — the throwaway execution absorbs CC firmware staging (and any
  first-execution AllGather corruption), so the returned result comes
  from a clean, already-staged execution.
- Host side only reshapes/transposes/slices for sharding and assembles the
  output; all FLOPs (blend, matmul, bias, ELU) run on device.
"""

import numpy as np

import concourse.mybir as mybir
import concourse.tile as tile
from concourse import bacc
from concourse.bass_utils import run_bass_kernel_spmd

N_CORES = 8
L = 3          # layers
E = 8          # experts
D = 2048       # feature dim
BATCH = 4096
BS = BATCH // N_CORES   # 512 batch rows per core
IS = D // N_CORES       # 256 contraction rows blended per core
KT = D // 128           # 16 k-tiles
MT = D // 128           # 16 m-tiles (output feature tiles)
HALF_M = MT // 2        # 8 psum banks per half

f32 = mybir.dt.float32
f32r = mybir.dt.float32r
bf16 = mybir.dt.bfloat16

USE_BF16 = True

_cache: dict = {}


def _build():
    nc = bacc.Bacc("TRN2", target_bir_lowering=False, debug=False,
                   num_devices=N_CORES)
    indt = bf16 if USE_BF16 else f32
    # Per-core inputs (pre-sharded/transposed by host):
    # WtT: (L, E, IS, D) = this core's i-slice of W transposed to [in, out]
    WtT = nc.dram_tensor("WtT", [L, E, IS, D], indt, kind="ExternalInput")
    # xT: (D, BS) = this core's batch shard, transposed
    xT = nc.dram_tensor("xT", [D, BS], indt, kind="ExternalInput")
    # cb: (128, E) = weight_blend broadcast across partitions
    cb = nc.dram_tensor("cb", [128, E], f32, kind="ExternalInput")
    # Bp: (128, L*MT) = biases; col l*MT+m holds B[l, m*128 : (m+1)*128]
    Bp = nc.dram_tensor("Bp", [128, L * MT], f32, kind="ExternalInput")
    # Output: yT (D, BS) = this core's output shard, transposed, bf16
    # (host converts back to f32; quantization adds ~2e-3 rel err).
    yT = nc.dram_tensor("yT", [D, BS], bf16, kind="ExternalOutput")

    with tile.TileContext(nc) as tc:
        with (
            tc.tile_pool(name="const", bufs=1) as cpool,
            tc.tile_pool(name="acts", bufs=1) as apool,
            tc.tile_pool(name="blend", bufs=1) as bpool,
            tc.tile_pool(name="exp", bufs=8) as epool,
            tc.tile_pool(name="wst", bufs=24) as wpool,
            tc.tile_pool(name="epi", bufs=4) as xpool,
            tc.tile_pool(name="psum", bufs=1, space="PSUM") as ppool,
            tc.tile_pool(name="dram", bufs=1, space="DRAM") as dram,
        ):
            cbt = cpool.tile([128, E], f32)
            bt = cpool.tile([128, L * MT], f32)
            nc.sync.dma_start(cbt[:], cb[:])
            nc.sync.dma_start(bt[:], Bp[:])

            # Tiny warmup AllGather: the first data-carrying CC op of an
            # execution pays a ~20us first-op cost (measured); burn it on
            # 32 bytes so AG0h0 runs at full rate. Its doorbell rings
            # immediately (only needs cb), so it also rides the runtime's
            # entry barrier. No completion-gating read: first-execution
            # staging corruption is absorbed by kernel()'s throwaway run.
            wuin = dram.tile([1, E], f32, name="wuin")
            wuout = dram.tile([N_CORES, E], f32, addr_space="Shared",
                              name="wuout")
            nc.sync.dma_start(wuin[:], cb[:1, :])
            nc.gpsimd.collective_compute(
                "AllGather", mybir.AluOpType.bypass,
                ins=[wuin.opt()], outs=[wuout.opt()],
                replica_groups=[list(range(N_CORES))],
            )

            mmdt = bf16 if USE_BF16 else f32r
            # Activations: two sets of KT tiles [128, BS], ping-pong.
            acts = [
                [apool.tile([128, BS], mmdt, name=f"act{s}_{k}")
                 for k in range(KT)]
                for s in range(2)
            ]
            # DRAM bounce buffers for the weight AllGathers. Each layer
            # gathers as two COLUMN halves (m 0:1024 / 1024:2048) that
            # align exactly with the matmul's m-half loop: half-AG a
            # lands ~30us before a monolithic AG would, so mm_l's first
            # m-half starts while half b is still gathering. This removes
            # the measured 9-15us stalls where monolithic AG1/AG2 ended
            # just after the matmuls needed them.
            agins = [
                [dram.tile([IS, D // 2], mmdt, name=f"agin{l}_{ch}")
                 for ch in range(2)]
                for l in range(L)
            ]
            agouts = [
                [dram.tile([D, D // 2], mmdt, addr_space="Shared",
                           name=f"agout{l}_{ch}")
                 for ch in range(2)]
                for l in range(L)
            ]

            # ---- blend + AllGather emission, one layer at a time.
            # Layers 0/1 go up-front; layer 2's blend is emitted between
            # mm0 and mm1 so its expert-load DMA traffic stays out of
            # AllGather-1's window. ----
            def emit_blend(l, ldeng=None, ltag="exp", lbufs=None):
                engs = ldeng if isinstance(ldeng, list) else [ldeng or nc.scalar]
                for h in range(IS // 128):  # 2 half-slices of 128 partitions
                    acc = bpool.tile([128, D], f32, name=f"acc{l}_{h}",
                                     tag=f"acc{h}", bufs=2)
                    accq = bpool.tile([128, D], mmdt, name=f"accq{l}_{h}",
                                      tag=f"accq{h}", bufs=2)
                    for e in range(E):
                        kw = {} if lbufs is None else {"bufs": lbufs}
                        et = epool.tile([128, D], indt, name=f"exp{l}_{h}_{e}",
                                        tag=ltag, **kw)
                        engs[(h * E + e) % len(engs)].dma_start(
                            et[:], WtT[l, e, h * 128:(h + 1) * 128, :]
                        )
                        if e == 0:
                            nc.vector.tensor_scalar_mul(
                                acc[:], et[:], cbt[:, 0:1]
                            )
                        elif e < E - 1:
                            # acc = (et * c_e) + acc
                            nc.vector.scalar_tensor_tensor(
                                acc[:], et[:], cbt[:, e:e + 1], acc[:],
                                mybir.AluOpType.mult, mybir.AluOpType.add,
                            )
                        else:
                            # last expert writes the matmul-dtype copy
                            nc.vector.scalar_tensor_tensor(
                                accq[:], et[:], cbt[:, e:e + 1], acc[:],
                                mybir.AluOpType.mult, mybir.AluOpType.add,
                            )
                    for ch in range(2):
                        nc.gpsimd.dma_start(
                            agins[l][ch][h * 128:(h + 1) * 128, :],
                            accq[:, ch * (D // 2):(ch + 1) * (D // 2)],
                        )
                for ch in range(2):
                    nc.gpsimd.collective_compute(
                        "AllGather", mybir.AluOpType.bypass,
                        ins=[agins[l][ch].opt()], outs=[agouts[l][ch].opt()],
                        replica_groups=[list(range(N_CORES))],
                    )

            emit_blend(0)
            # x loads go first on the sync queue: emitted later they land
            # inside AllGather-0's window and slow it (queue is FIFO).
            for k in range(KT):
                xsrc = xT[k * 128:(k + 1) * 128, :]
                nc.sync.dma_start(
                    acts[0][k][:], xsrc if USE_BF16 else xsrc.bitcast(f32r)
                )
            emit_blend(1, ldeng=nc.sync, ltag="exp1", lbufs=4)
            # blend2 emitted up-front too: its DVE ops must precede mm0's
            # epilogue ops in the Vector engine's in-order queue, else
            # AG2's doorbell waits on mm0's full epilogue (engine queues
            # are FIFO; DMA loads schedule eagerly either way).
            # blend2's loads split across both HWDGE queues: they queue
            # behind blend0 (scalar) / x+blend1 (sync) FIFO entries and
            # finish ~75us — before AllGather-0's window opens (~88us).
            # (gpsimd/SWDGE tried instead: too slow, loads leaked into
            # AG0's window and cost it ~18us.)
            emit_blend(2, ldeng=[nc.scalar, nc.sync], ltag="exp2", lbufs=4)

            for l in range(L):
                # ---- matmul: y_l.T[m,:] = sum_k w_l.T[k,m].T @ act[k] ----
                src = acts[l % 2]
                dst = acts[(l + 1) % 2]
                # Quarter-groups of 4 m-tiles on ALTERNATING psum bank
                # sets (0-3 / 4-7): group g+1's matmuls don't share banks
                # with group g, so g's epilogue drains under g+1's compute
                # instead of stalling the PE ~3us at every boundary.
                QM = HALF_M // 2
                for q in range(4):
                    ch, sub = q // 2, q % 2
                    psums = [
                        ppool.tile([128, BS], f32, name=f"ps{l}_{q}_{m}",
                                   tag=f"bank{(q % 2) * QM + m}")
                        for m in range(QM)
                    ]
                    for k in range(KT):
                        ws = wpool.tile([128, QM * 128], mmdt,
                                        name=f"ws{l}_{q}_{k}", tag="ws")
                        # alternate ws loads across both HWDGE queues
                        # (scalar queue is idle after the head phase) to
                        # halve per-tile feed latency under AG contention
                        wse = nc.sync if k % 2 == 0 else nc.scalar
                        wse.dma_start(
                            ws[:],
                            agouts[l][ch][k * 128:(k + 1) * 128,
                                          sub * QM * 128:(sub + 1) * QM * 128],
                        )
                        for m in range(QM):
                            nc.tensor.matmul(
                                psums[m][:],
                                ws[:, m * 128:(m + 1) * 128],
                                src[k][:],
                                start=(k == 0),
                                stop=(k == KT - 1),
                            )
                    # ---- epilogue: bias (+ ELU), write next-layer acts ----
                    for m in range(QM):
                        gm = q * QM + m
                        bias = bt[:, l * MT + gm: l * MT + gm + 1]
                        ps = psums[m]
                        if l < L - 1:
                            tt = xpool.tile([128, BS], f32,
                                            name=f"t{l}_{gm}", tag="tmin")
                            zt = xpool.tile([128, BS], f32,
                                            name=f"z{l}_{gm}", tag="zbias")
                            ut = xpool.tile([128, BS], f32,
                                            name=f"u{l}_{gm}", tag="uexp")
                            # t = min(psum + bias, 0) on DVE
                            nc.vector.tensor_scalar(
                                tt[:], ps[:], bias, 0.0,
                                mybir.AluOpType.add, mybir.AluOpType.min,
                            )
                            # z = psum + bias on ACT
                            nc.scalar.activation(
                                zt[:], ps[:],
                                mybir.ActivationFunctionType.Identity,
                                bias=bias,
                            )
                            # u = exp(t) on ACT
                            nc.scalar.activation(
                                ut[:], tt[:], mybir.ActivationFunctionType.Exp
                            )
                            # act_next = max(u - 1, z) on DVE, in matmul dtype
                            nc.vector.scalar_tensor_tensor(
                                dst[gm][:], ut[:], 1.0, zt[:],
                                mybir.AluOpType.subtract, mybir.AluOpType.max,
                            )
                        else:
                            ot = xpool.tile([128, BS], bf16,
                                            name=f"o{gm}", tag="outt")
                            # bias-add split across ACT and DVE so the
                            # final half's serial epilogue chain halves.
                            if gm % 2 == 0:
                                nc.scalar.activation(
                                    ot[:], ps[:],
                                    mybir.ActivationFunctionType.Identity,
                                    bias=bias,
                                )
                            else:
                                nc.vector.tensor_scalar_add(
                                    ot[:], ps[:], bias
                                )
                            # HWDGE store (~0.6us fixed vs ~2us SWDGE);
                            # alternate queues so the 16 stores drain in
                            # parallel at the tail.
                            steng = nc.scalar if gm % 2 == 0 else nc.gpsimd
                            steng.dma_start(
                                yT[gm * 128:(gm + 1) * 128, :], ot[:]
                            )
    nc.finalize()
    return nc


def _get_nc():
    if "nc" not in _cache:
        _cache["nc"] = _build()
    return _cache["nc"]


def make_in_maps(weight_blend, x, W, B):
    weight_blend = np.asarray(weight_blend, dtype=np.float32)
    x = np.asarray(x, dtype=np.float32)
    W = np.asarray(W, dtype=np.float32)
    B = np.asarray(B, dtype=np.float32)

    cb = np.ascontiguousarray(np.broadcast_to(weight_blend[None, :], (128, E)))
    # Bp[p, l*MT+m] = B[l, m*128+p]
    Bp = np.ascontiguousarray(
        B.reshape(L, MT, 128).transpose(2, 0, 1).reshape(128, L * MT)
    )

    import ml_dtypes
    indt = ml_dtypes.bfloat16 if USE_BF16 else np.float32
    in_maps = []
    for k in range(N_CORES):
        WtT = np.ascontiguousarray(
            W[:, :, :, k * IS:(k + 1) * IS].transpose(0, 1, 3, 2)
        ).astype(indt)
        xTk = np.ascontiguousarray(x[k * BS:(k + 1) * BS, :].T).astype(indt)
        in_maps.append({"WtT": WtT, "xT": xTk, "cb": cb, "Bp": Bp})
    return in_maps


def kernel(weight_blend, x, W, B) -> np.ndarray:
    in_maps = make_in_maps(weight_blend, x, W, B)
    nc = _get_nc()
    last_err = None
    for attempt in range(3):
        try:
            if "staged" not in _cache:
                # Throwaway execution: absorbs CC firmware staging — the
                # first execution's collectives can be corrupted by
                # doorbells ringing during staging, so never return it.
                run_bass_kernel_spmd(nc, in_maps,
                                     core_ids=list(range(N_CORES)))
                _cache["staged"] = True
            res = run_bass_kernel_spmd(nc, in_maps,
                                       core_ids=list(range(N_CORES)))
            out = np.empty((BATCH, D), dtype=np.float32)
            for k in range(N_CORES):
                out[k * BS:(k + 1) * BS, :] = res.results[k]["yT"].T.astype(
                    np.float32)
            if np.isfinite(out).all():
                return out
            last_err = RuntimeError("non-finite kernel output")
        except Exception as e:  # transient NRT/device wedge: retry
            last_err = e
        import time as _time
        _time.sleep(10 * (attempt + 1))
    raise last_err
